# revision 29
# baseline (speedup 1.0000x reference)
"""Trainium2 Bass kernel for nn_MoEsparseRoutingForClassification.

Reference computation (B=64, S=128, H=1024, E=8, L=2):
    x = X[:, 0, :]                                   # CLS token [B,H]
    y[b,o]   = sum_e g[b,e] * (x[b] . dense_w[e,o,:]) + (g @ dense_b)[b,o]
    t        = tanh(y)
    out[b,l] = sum_e g[b,e] * (t[b] . out_w[e,l,:])  + (g @ out_b)[b,l]

Distribution: the H output dim of the dense layer is sharded 8 ways
(OC=128 per core).  Core c computes y[:, c*OC:(c+1)*OC] (which needs the
full CLS token but only a slice dense_w[:, c_slice, :]), applies tanh,
and contracts its slice against out_w[:, :, c_slice] to produce a
partial [L,128] logit block.  The partials (incl. the out_b bias, fed
only to core 0) sum to the full output on the host.  No cross-core
collective is needed.

The dense_w stream is fp8 e3m4 (4 mantissa bits, host-scaled by 128
into its normal range; 1/128 folded into the gate consts, 128 into
dense_b) - a quarter of the fp32 bytes.  Everything else feeding the
PE is bf16; PSUM accumulation stays fp32.  rel-err budget is 2e-2;
measured 1.63e-2 scaled-max / 1.60e-2 rel-L2, deterministic for the
seeded inputs (verified bit-identical against a numpy simulation of
the quantization chain).

DMA: one ring (sync), ordered so each chain's completion unblocks work
just in time (DMA engines drain descriptor chains mostly in doorbell
order; doorbell->data ~1.5us, dma-complete->sem-visible ~0.3-0.7us,
~23 GB/s per engine x 16 engines ~ 300 GB/s aggregate):
  cp (34 KB const pack: gates/out_w/biases/gate-broadcast consts, one
  sem for everything the small matmuls need) | wx = xt|w1 combined
  stream in 3 chunks (xt+k0-3 / k4-6 / k7) that the PE chases; the
  last chunk is a single k-tile so only ~0.7us of PE work remains
  after the final bytes land.

PE program order: dense_b fold (K=1 matmul into psum_y, so the bias
needs no DVE add), sel_ow, out_b seed into the [2,128] output psum,
gate-broadcast table (K=8 matmul, replaces a 128-row gc DMA + 8 DVE
builds), then the 16 chunk-chasing stage-1 matmuls.  Post-mix chain:
mult + 3 tree adds + tanh + 2 accum-dots, then a PE transpose
accumulates the [128,2] result onto the out_b seed so the output DMA
is 2 rows x 512 B.

Everything arithmetic runs on device; the host only slices, transposes
(layout prep), and sums the partial outputs.
"""

import sys

import numpy as np

for _p in ("/opt/trn_rl_repo",):
    if _p not in sys.path:
        sys.path.insert(0, _p)

# If the environment sets BASS_TRACE but lacks antenv.axon_hooks (this agent
# image does), run_bass_kernel_spmd would crash on import; pre-seed a no-op
# module so tracing degrades gracefully instead.
try:  # pragma: no cover
    import antenv.axon_hooks  # noqa: F401
except Exception:  # pragma: no cover
    import types as _types

    _m = _types.ModuleType("antenv.axon_hooks")
    _m._hook = None
    _m.set_axon_ntff_profile_hook = lambda h: setattr(_m, "_hook", h)
    _m.get_axon_ntff_profile_hook = lambda: _m._hook
    sys.modules["antenv.axon_hooks"] = _m

B, S, H = 64, 128, 1024
E, L = 8, 2
NCORES = 8
OC = H // NCORES          # dense-output slice per core (128)
HC = OC // 2              # half-slice mapped to a PSUM partition half (64)
KT = H // 128             # contraction tiles
P = 128

# w1 travels as fp8 e3m4 (4 mantissa bits), host-scaled by WSCALE into its
# normal range [2^-2, 15.5]; 1/WSCALE is folded into the gate-broadcast
# consts and WSCALE into dense_b, so no extra device work is needed.
WSCALE = 128.0
# fp8 rows are thin (1 KB per k-tile per partition) and DMA efficiency
# drops fast below ~3 KB rows; bigger rows also move faster per byte, so
# front-load the first chunk (earlier PE start) and keep the second
# large enough that its sem lands before the PE needs it.
W1_CHUNKS = ((0, 5), (5, 8))   # k-tile chunk boundaries

# const-pack layout (bf16, [E, CPK]); row 0 additionally carries dense_b
OGT = 0                       # gates.T [E, B]
OOW = OGT + B                 # ow2 [E, 2*L*HC]
OOB = OOW + L * OC            # out_b [E, L] (zeros except core 0)
OGTZ = OOB + L                # gates.T | zeros [E, P]
OGTD = OGTZ + P               # gates.T | gates.T [E, P]
OEBC = OGTD + P               # kron(I_E, ones[HC]) [E, E*HC]
ODB = OEBC + E * HC           # dense_b row (row 0 only) [1, 2*E*HC]
CPK = ODB + 2 * E * HC

_cached = None


def _build():
    from contextlib import ExitStack

    import concourse.tile as tile
    from concourse import bacc, mybir

    F32 = mybir.dt.float32
    BF16 = mybir.dt.bfloat16
    AF = mybir.ActivationFunctionType
    OP = mybir.AluOpType

    nc = bacc.Bacc("TRN2", target_bir_lowering=False, debug=False,
                   num_devices=NCORES)

    F8 = mybir.dt.float8e3
    xt_d = nc.dram_tensor("xt", [P, KT, B], BF16, kind="ExternalInput")
    w1_d = nc.dram_tensor("w1", [P, KT, 2, E, HC], F8, kind="ExternalInput")
    cp_d = nc.dram_tensor("cp", [E, CPK], BF16, kind="ExternalInput")
    out_d = nc.dram_tensor("out", [L, P], F32, kind="ExternalOutput")

    with tile.TileContext(nc) as tc, ExitStack() as ctx:
        consts = ctx.enter_context(tc.tile_pool(name="consts", bufs=1))
        wpool = ctx.enter_context(tc.tile_pool(name="wpool", bufs=1))
        mixp = ctx.enter_context(tc.tile_pool(name="mixp", bufs=2))
        smallp = ctx.enter_context(tc.tile_pool(name="smallp", bufs=1))
        psy = ctx.enter_context(tc.tile_pool(name="psy", bufs=1, space="PSUM"))
        pss = ctx.enter_context(tc.tile_pool(name="pss", bufs=1, space="PSUM"))
        psg = ctx.enter_context(tc.tile_pool(name="psg", bufs=1, space="PSUM"))

        # Sync-ring order = DMA chain service order.  xt first: it gates
        # the k-stream, while cp's consumers (the small matmuls) have
        # slack until chunk 0 lands.
        xt_t = consts.tile([P, KT, B], BF16)
        nc.sync.dma_start(out=xt_t, in_=xt_d.ap())
        cp_t = consts.tile([E, CPK], BF16)
        nc.sync.dma_start(out=cp_t, in_=cp_d.ap())
        w1_t = wpool.tile([P, KT, 2, E, HC], F8)
        for klo, khi in W1_CHUNKS:
            nc.sync.dma_start(
                out=w1_t[:, klo:khi],
                in_=w1_d.ap()[:, klo:khi],
            )

        gt_t = cp_t[:, OGT:OGT + B]
        ow_t = cp_t[:, OOW:OOW + L * OC].rearrange(
            "e (h l c) -> e h l c", h=2, l=L)
        ob_t = cp_t[:, OOB:OOB + L]
        gtz_t = cp_t[:, OGTZ:OGTZ + P]
        gtdup_t = cp_t[:, OGTD:OGTD + P]
        ebc_t = cp_t[:, OEBC:OEBC + E * HC]

        # ---- dense_b fold: psum_y[64h+b, (e,c)] starts at db[e,c] ----
        ones1 = smallp.tile([1, B], BF16)
        nc.vector.memset(ones1[:], 1.0)
        psum_y = psy.tile([P, E, HC], F32)
        for h in range(2):
            nc.tensor.matmul(
                psum_y[h * 64:h * 64 + 64, :, :].rearrange("b e c -> b (e c)"),
                ones1[:],
                cp_t[0:1, ODB + h * E * HC:ODB + (h + 1) * E * HC],
                start=True, stop=False, skip_group_check=True,
            )

        # ---- small matmuls (all bf16, one DMA sem) ----
        # sel_ow^h [64h+b, (l, hc)]
        psum_ow = pss.tile([P, L, HC], F32)
        for h in range(2):
            sl = slice(h * 64, h * 64 + 64)
            nc.tensor.matmul(
                psum_ow[sl, :, :].rearrange("b l c -> b (l c)"),
                gt_t, ow_t[:, h].rearrange("e l c -> e (l c)"),
                start=True, stop=True, skip_group_check=True,
            )
        # Output accumulator [l, p]: seed with sel_ob^T (only core 0
        # carries real ob); the stage-2 transpose accumulates on top.
        psum_o2 = pss.tile([L, P], F32)
        nc.tensor.matmul(psum_o2[:], ob_t, gtz_t,
                         start=True, stop=False, skip_group_check=True)
        # Gate-broadcast table gb[p, (e, hc)] = g[b, e] via one K=8 matmul.
        psum_gb = psg.tile([P, E, HC], F32)
        nc.tensor.matmul(psum_gb[:, :, :].rearrange("p e c -> p (e c)"),
                         gtdup_t, ebc_t, start=True, stop=True)
        gb_t = consts.tile([P, E, HC], F32)
        nc.vector.tensor_copy(gb_t[:], psum_gb[:])

        # Identity for the final PE transpose, built on the idle gpsimd.
        onesq = smallp.tile([P, P], F32)
        nc.gpsimd.memset(onesq[:], 1.0)
        idt_t = consts.tile([P, P], F32)
        nc.gpsimd.affine_select(
            out=idt_t[:], in_=onesq[:], pattern=[[-1, P]],
            compare_op=OP.is_equal, fill=0.0, base=0, channel_multiplier=1,
        )

        # ---- stage 1: y[64h+b, (e, hc)] += x . dense_w[e, oc_half, :] ----
        # k-outer so the PE consumes each w1 chunk as it lands.
        # NOTE: splitting the last k-tile into e-halves (partial-width
        # stop matmuls) hard-faults the PE (NRT_EXEC_UNIT_UNRECOVERABLE);
        # keep full-width accumulation.
        for k in range(KT):
            for h in range(2):
                nc.tensor.matmul(
                    psum_y[h * 64:h * 64 + 64, :, :].rearrange(
                        "b e c -> b (e c)"),
                    xt_t[:, k, :],
                    w1_t[:, k, h].rearrange("p e c -> p (e c)"),
                    start=False,
                    stop=(k == KT - 1),
                    skip_group_check=True,
                )

        # bf16 tree intermediates: the mult reads fp32 PSUM but writes
        # bf16, and the adds then run at 2x DVE throughput.
        prod_t = mixp.tile([P, E, HC], BF16)
        nc.vector.tensor_tensor(
            out=prod_t[:], in0=psum_y[:], in1=gb_t[:], op=OP.mult,
        )
        # contiguous pairwise tree over e (strided reduce is ~2x slower)
        t1 = mixp.tile([P, 4, HC], BF16)
        nc.vector.tensor_add(t1[:], prod_t[:, 0:4, :], prod_t[:, 4:8, :])
        t2 = mixp.tile([P, 2, HC], BF16)
        nc.vector.tensor_add(t2[:], t1[:, 0:2, :], t1[:, 2:4, :])
        t3 = mixp.tile([P, HC], BF16)
        nc.vector.tensor_add(t3[:], t2[:, 0, :], t2[:, 1, :])

        t_t = smallp.tile([P, HC], F32)
        nc.scalar.activation(t_t[:], t3[:], AF.Tanh)

        # ---- stage 2: pre[64h+b, l] = sum_hc t * sel_ow ----
        # NOTE: InstTensorTensorReduce faults TRN2; scalar_tensor_tensor with
        # accum_out (free-dim sum) is the reliable path.
        pre_t = smallp.tile([P, L], F32)
        dump = smallp.tile([P, HC], F32)
        for l in range(L):
            nc.vector.scalar_tensor_tensor(
                out=dump[:],
                in0=psum_ow[:, l, :],
                scalar=1.0,
                in1=t_t[:],
                op0=OP.mult,
                op1=OP.mult,
                accum_out=pre_t[:, l:l + 1],
            )
        # PE transpose [128,2] -> [2,128], accumulating onto the ob seed.
        nc.tensor.matmul(psum_o2[:], pre_t[:], idt_t[:],
                         is_transpose=True, start=False, stop=True,
                         skip_group_check=True)
        o2_t = smallp.tile([L, P], F32)
        nc.vector.tensor_copy(o2_t[:], psum_o2[:])

        nc.sync.dma_start(out=out_d.ap(), in_=o2_t[:])

    nc.compile()
    return nc


def _prep_inputs(X, gates, dense_w, dense_b, out_w, out_b):
    """Host-side layout prep (slice/transpose/cast only) -> per-core maps."""
    import ml_dtypes

    BF = ml_dtypes.bfloat16
    X = np.asarray(X, dtype=np.float32)
    gates = np.asarray(gates, dtype=np.float32)
    dense_w = np.asarray(dense_w, dtype=np.float32)
    dense_b = np.asarray(dense_b, dtype=np.float32)
    out_w = np.asarray(out_w, dtype=np.float32)
    out_b = np.asarray(out_b, dtype=np.float32)

    xcls = X[:, 0, :]                                     # [B, H]
    # xt[i_lo, k, b] = x[b, k*128 + i_lo]
    xt = np.ascontiguousarray(
        xcls.T.reshape(KT, P, B).transpose(1, 0, 2)).astype(BF)
    gt = np.ascontiguousarray(gates.T)                    # [E, B]
    gtz = np.concatenate([gt, np.zeros_like(gt)], axis=1)  # [E, 128]
    # gb carries 1/WSCALE to undo the fp8 weight scaling during the mix
    gtdup = np.concatenate([gt, gt], axis=1) / WSCALE      # [E, 128]
    ebc = np.kron(np.eye(E, dtype=np.float32),
                  np.ones((1, HC), dtype=np.float32))      # [E, E*HC]

    in_maps = []
    for c in range(NCORES):
        sl = slice(c * OC, (c + 1) * OC)
        # w1[i_lo, k, h, e, hc] = dense_w[e, c*OC + h*64 + hc, k*128 + i_lo]
        w1 = np.ascontiguousarray(
            (dense_w[:, sl, :] * WSCALE)        # [E, OC, H], fp8-ranged
            .reshape(E, 2, HC, KT, P)           # [e, h, hc, k, i_lo]
            .transpose(4, 3, 1, 0, 2)           # [i_lo, k, h, e, hc]
        ).astype(ml_dtypes.float8_e3m4)

        # db row: dense_b[e, c*OC + h*64 + hc] laid as (h, e, hc); row 0
        # only.  Carries WSCALE so it accumulates into the scaled psum.
        dbrow = np.zeros((E, 2 * E * HC), dtype=np.float32)
        dbrow[0] = (dense_b[:, sl].reshape(E, 2, HC)
                    .transpose(1, 0, 2).reshape(-1)) * WSCALE

        # ow2[e, (h, l, hc)] = out_w[e, l, c*OC + h*64 + hc]
        ow2 = (out_w[:, :, sl].reshape(E, L, 2, HC)
               .transpose(0, 2, 1, 3).reshape(E, L * OC))
        ob = out_b if c == 0 else np.zeros_like(out_b)
        cp = np.ascontiguousarray(
            np.concatenate([gt, ow2, ob, gtz, gtdup, ebc, dbrow], axis=1)
        ).astype(BF)
        in_maps.append({
            "xt": xt,
            "w1": w1,
            "cp": cp,
        })
    return in_maps


def _run(in_maps, trace=False, tmpdir=None):
    global _cached
    from concourse.bass_utils import run_bass_kernel_spmd

    if _cached is None:
        _cached = _build()
    res = run_bass_kernel_spmd(
        _cached, in_maps, list(range(NCORES)), trace=trace, tmpdir=tmpdir,
    )
    return res


def kernel(X, gates, dense_w, dense_b, out_w, out_b):
    in_maps = _prep_inputs(X, gates, dense_w, dense_b, out_w, out_b)
    res = _run(in_maps)
    acc = np.zeros((B, L), dtype=np.float64)
    for c in range(NCORES):
        part = res.results[c]["out"].astype(np.float64)   # [L, 128]
        acc += part.T.reshape(2, B, L).sum(axis=0)
    return acc.astype(np.float32)


# revision 30
# speedup vs baseline: 1.1398x; 1.1398x over previous
"""Trainium2 Bass kernel for nn_MoEsparseRoutingForClassification.

Reference computation (B=64, S=128, H=1024, E=8, L=2):
    x = X[:, 0, :]                                   # CLS token [B,H]
    y[b,o]   = sum_e g[b,e] * (x[b] . dense_w[e,o,:]) + (g @ dense_b)[b,o]
    t        = tanh(y)
    out[b,l] = sum_e g[b,e] * (t[b] . out_w[e,l,:])  + (g @ out_b)[b,l]

Distribution: the H output dim of the dense layer is sharded 8 ways
(OC=128 per core).  Core c computes y[:, c*OC:(c+1)*OC] (which needs the
full CLS token but only a slice dense_w[:, c_slice, :]), applies tanh,
and contracts its slice against out_w[:, :, c_slice] to produce a
partial [L,128] logit block.  The partials (incl. the out_b bias, fed
only to core 0) sum to the full output on the host.  No cross-core
collective is needed.

The dense_w stream is fp8 e3m4 (4 mantissa bits, host-scaled by 128
into its normal range; 1/128 folded into the gate consts, 128 into
dense_b) - a quarter of the fp32 bytes.  Everything else feeding the
PE is bf16; PSUM accumulation stays fp32.  rel-err budget is 2e-2;
measured 1.63e-2 scaled-max / 1.60e-2 rel-L2, deterministic for the
seeded inputs (verified bit-identical against a numpy simulation of
the quantization chain).

DMA: one ring (sync), ordered so each chain's completion unblocks work
just in time (DMA engines drain descriptor chains mostly in doorbell
order; doorbell->data ~1.5us, dma-complete->sem-visible ~0.3-0.7us,
~23 GB/s per engine x 16 engines ~ 300 GB/s aggregate):
  cp (34 KB const pack: gates/out_w/biases/gate-broadcast consts, one
  sem for everything the small matmuls need) | wx = xt|w1 combined
  stream in 3 chunks (xt+k0-3 / k4-6 / k7) that the PE chases; the
  last chunk is a single k-tile so only ~0.7us of PE work remains
  after the final bytes land.

PE program order: dense_b fold (K=1 matmul into psum_y, so the bias
needs no DVE add), sel_ow, out_b seed into the [2,128] output psum,
gate-broadcast table (K=8 matmul, replaces a 128-row gc DMA + 8 DVE
builds), then the 16 chunk-chasing stage-1 matmuls.  Post-mix chain:
mult + 3 tree adds + tanh + 2 accum-dots, then a PE transpose
accumulates the [128,2] result onto the out_b seed so the output DMA
is 2 rows x 512 B.

Everything arithmetic runs on device; the host only slices, transposes
(layout prep), and sums the partial outputs.
"""

import sys

import numpy as np

for _p in ("/opt/trn_rl_repo",):
    if _p not in sys.path:
        sys.path.insert(0, _p)

# If the environment sets BASS_TRACE but lacks antenv.axon_hooks (this agent
# image does), run_bass_kernel_spmd would crash on import; pre-seed a no-op
# module so tracing degrades gracefully instead.
try:  # pragma: no cover
    import antenv.axon_hooks  # noqa: F401
except Exception:  # pragma: no cover
    import types as _types

    _m = _types.ModuleType("antenv.axon_hooks")
    _m._hook = None
    _m.set_axon_ntff_profile_hook = lambda h: setattr(_m, "_hook", h)
    _m.get_axon_ntff_profile_hook = lambda: _m._hook
    sys.modules["antenv.axon_hooks"] = _m

B, S, H = 64, 128, 1024
E, L = 8, 2
NCORES = 8
OC = H // NCORES          # dense-output slice per core (128)
HC = OC // 2              # half-slice mapped to a PSUM partition half (64)
KT = H // 128             # contraction tiles
P = 128

# w1 travels as fp8 e3m4 (4 mantissa bits), host-scaled by WSCALE into its
# normal range [2^-2, 15.5]; 1/WSCALE is folded into the gate-broadcast
# consts and WSCALE into dense_b, so no extra device work is needed.
WSCALE = 128.0
# fp8 rows are thin (1 KB per k-tile per partition) and DMA efficiency
# drops fast below ~3 KB rows: two symmetric 4-tile chunks (4-KB rows)
# measured best -- (5,3) and (2,3,3) splits both regress (the smaller-
# row chunk gets poor engine service and dribbles).
W1_CHUNKS = ((0, 4), (4, 8))   # k-tile chunk boundaries

# const-pack layout (bf16, [E, CPK]); row 0 additionally carries dense_b
OGT = 0                       # gates.T [E, B]
OOW = OGT + B                 # ow2 [E, 2*L*HC]
OOB = OOW + L * OC            # out_b [E, L] (zeros except core 0)
OGTZ = OOB + L                # gates.T | zeros [E, P]
OGTD = OGTZ + P               # gates.T | gates.T [E, P]
OEBC = OGTD + P               # kron(I_E, ones[HC]) [E, E*HC]
ODB = OEBC + E * HC           # dense_b row (row 0 only) [1, 2*E*HC]
CPK = ODB + 2 * E * HC

_cached = None


def _build():
    from contextlib import ExitStack

    import concourse.tile as tile
    from concourse import bacc, mybir

    F32 = mybir.dt.float32
    BF16 = mybir.dt.bfloat16
    AF = mybir.ActivationFunctionType
    OP = mybir.AluOpType

    nc = bacc.Bacc("TRN2", target_bir_lowering=False, debug=False,
                   num_devices=NCORES)

    F8 = mybir.dt.float8e3
    xt_d = nc.dram_tensor("xt", [P, KT, B], BF16, kind="ExternalInput")
    w1_d = nc.dram_tensor("w1", [P, KT, 2, E, HC], F8, kind="ExternalInput")
    cp_d = nc.dram_tensor("cp", [E, CPK], BF16, kind="ExternalInput")
    out_d = nc.dram_tensor("out", [L, P], F32, kind="ExternalOutput")

    with tile.TileContext(nc) as tc, ExitStack() as ctx:
        consts = ctx.enter_context(tc.tile_pool(name="consts", bufs=1))
        wpool = ctx.enter_context(tc.tile_pool(name="wpool", bufs=1))
        mixp = ctx.enter_context(tc.tile_pool(name="mixp", bufs=2))
        smallp = ctx.enter_context(tc.tile_pool(name="smallp", bufs=1))
        psy = ctx.enter_context(tc.tile_pool(name="psy", bufs=1, space="PSUM"))
        pss = ctx.enter_context(tc.tile_pool(name="pss", bufs=1, space="PSUM"))
        psg = ctx.enter_context(tc.tile_pool(name="psg", bufs=1, space="PSUM"))

        # Sync-ring order = DMA chain service order.  xt first: it gates
        # the k-stream, while cp's consumers (the small matmuls) have
        # slack until chunk 0 lands.
        xt_t = consts.tile([P, KT, B], BF16)
        nc.sync.dma_start(out=xt_t, in_=xt_d.ap())
        cp_t = consts.tile([E, CPK], BF16)
        nc.sync.dma_start(out=cp_t, in_=cp_d.ap())
        w1_t = wpool.tile([P, KT, 2, E, HC], F8)
        for klo, khi in W1_CHUNKS:
            nc.sync.dma_start(
                out=w1_t[:, klo:khi],
                in_=w1_d.ap()[:, klo:khi],
            )

        gt_t = cp_t[:, OGT:OGT + B]
        ow_t = cp_t[:, OOW:OOW + L * OC].rearrange(
            "e (h l c) -> e h l c", h=2, l=L)
        ob_t = cp_t[:, OOB:OOB + L]
        gtz_t = cp_t[:, OGTZ:OGTZ + P]
        gtdup_t = cp_t[:, OGTD:OGTD + P]
        ebc_t = cp_t[:, OEBC:OEBC + E * HC]

        # ---- dense_b fold: psum_y[64h+b, (e,c)] starts at db[e,c] ----
        ones1 = smallp.tile([1, B], BF16)
        nc.vector.memset(ones1[:], 1.0)
        psum_y = psy.tile([P, E, HC], F32)
        for h in range(2):
            nc.tensor.matmul(
                psum_y[h * 64:h * 64 + 64, :, :].rearrange("b e c -> b (e c)"),
                ones1[:],
                cp_t[0:1, ODB + h * E * HC:ODB + (h + 1) * E * HC],
                start=True, stop=False, skip_group_check=True,
            )

        # ---- small matmuls (all bf16, one DMA sem) ----
        # sel_ow^h [64h+b, (l, hc)]
        psum_ow = pss.tile([P, L, HC], F32)
        for h in range(2):
            sl = slice(h * 64, h * 64 + 64)
            nc.tensor.matmul(
                psum_ow[sl, :, :].rearrange("b l c -> b (l c)"),
                gt_t, ow_t[:, h].rearrange("e l c -> e (l c)"),
                start=True, stop=True, skip_group_check=True,
            )
        # Output accumulator [l, p]: seed with sel_ob^T (only core 0
        # carries real ob); the stage-2 transpose accumulates on top.
        psum_o2 = pss.tile([L, P], F32)
        nc.tensor.matmul(psum_o2[:], ob_t, gtz_t,
                         start=True, stop=False, skip_group_check=True)
        # Gate-broadcast table gb[p, (e, hc)] = g[b, e] via one K=8 matmul.
        psum_gb = psg.tile([P, E, HC], F32)
        nc.tensor.matmul(psum_gb[:, :, :].rearrange("p e c -> p (e c)"),
                         gtdup_t, ebc_t, start=True, stop=True)
        gb_t = consts.tile([P, E, HC], F32)
        nc.vector.tensor_copy(gb_t[:], psum_gb[:])

        # Identity for the final PE transpose, built on the idle gpsimd.
        onesq = smallp.tile([P, P], F32)
        nc.gpsimd.memset(onesq[:], 1.0)
        idt_t = consts.tile([P, P], F32)
        nc.gpsimd.affine_select(
            out=idt_t[:], in_=onesq[:], pattern=[[-1, P]],
            compare_op=OP.is_equal, fill=0.0, base=0, channel_multiplier=1,
        )

        # ---- stage 1: y[64h+b, (e, hc)] += x . dense_w[e, oc_half, :] ----
        # k-outer so the PE consumes each w1 chunk as it lands.
        # NOTE: splitting the last k-tile into e-halves (partial-width
        # stop matmuls) hard-faults the PE (NRT_EXEC_UNIT_UNRECOVERABLE);
        # keep full-width accumulation.
        for k in range(KT):
            for h in range(2):
                nc.tensor.matmul(
                    psum_y[h * 64:h * 64 + 64, :, :].rearrange(
                        "b e c -> b (e c)"),
                    xt_t[:, k, :],
                    w1_t[:, k, h].rearrange("p e c -> p (e c)"),
                    start=False,
                    stop=(k == KT - 1),
                    skip_group_check=True,
                )

        # bf16 tree intermediates: the mult reads fp32 PSUM but writes
        # bf16, and the adds then run at 2x DVE throughput.
        prod_t = mixp.tile([P, E, HC], BF16)
        nc.vector.tensor_tensor(
            out=prod_t[:], in0=psum_y[:], in1=gb_t[:], op=OP.mult,
        )
        # contiguous pairwise tree over e (strided reduce is ~2x slower)
        t1 = mixp.tile([P, 4, HC], BF16)
        nc.vector.tensor_add(t1[:], prod_t[:, 0:4, :], prod_t[:, 4:8, :])
        t2 = mixp.tile([P, 2, HC], BF16)
        nc.vector.tensor_add(t2[:], t1[:, 0:2, :], t1[:, 2:4, :])
        t3 = mixp.tile([P, HC], BF16)
        nc.vector.tensor_add(t3[:], t2[:, 0, :], t2[:, 1, :])

        t_t = smallp.tile([P, HC], F32)
        nc.scalar.activation(t_t[:], t3[:], AF.Tanh)

        # ---- stage 2: pre[64h+b, l] = sum_hc t * sel_ow ----
        # NOTE: InstTensorTensorReduce faults TRN2; scalar_tensor_tensor with
        # accum_out (free-dim sum) is the reliable path.
        pre_t = smallp.tile([P, L], F32)
        dump = smallp.tile([P, HC], F32)
        for l in range(L):
            nc.vector.scalar_tensor_tensor(
                out=dump[:],
                in0=psum_ow[:, l, :],
                scalar=1.0,
                in1=t_t[:],
                op0=OP.mult,
                op1=OP.mult,
                accum_out=pre_t[:, l:l + 1],
            )
        # PE transpose [128,2] -> [2,128], accumulating onto the ob seed.
        nc.tensor.matmul(psum_o2[:], pre_t[:], idt_t[:],
                         is_transpose=True, start=False, stop=True,
                         skip_group_check=True)
        o2_t = smallp.tile([L, P], F32)
        nc.vector.tensor_copy(o2_t[:], psum_o2[:])

        nc.sync.dma_start(out=out_d.ap(), in_=o2_t[:])

    nc.compile()
    return nc


def _prep_inputs(X, gates, dense_w, dense_b, out_w, out_b):
    """Host-side layout prep (slice/transpose/cast only) -> per-core maps."""
    import ml_dtypes

    BF = ml_dtypes.bfloat16
    X = np.asarray(X, dtype=np.float32)
    gates = np.asarray(gates, dtype=np.float32)
    dense_w = np.asarray(dense_w, dtype=np.float32)
    dense_b = np.asarray(dense_b, dtype=np.float32)
    out_w = np.asarray(out_w, dtype=np.float32)
    out_b = np.asarray(out_b, dtype=np.float32)

    xcls = X[:, 0, :]                                     # [B, H]
    # xt[i_lo, k, b] = x[b, k*128 + i_lo]
    xt = np.ascontiguousarray(
        xcls.T.reshape(KT, P, B).transpose(1, 0, 2)).astype(BF)
    gt = np.ascontiguousarray(gates.T)                    # [E, B]
    gtz = np.concatenate([gt, np.zeros_like(gt)], axis=1)  # [E, 128]
    # gb carries 1/WSCALE to undo the fp8 weight scaling during the mix
    gtdup = np.concatenate([gt, gt], axis=1) / WSCALE      # [E, 128]
    ebc = np.kron(np.eye(E, dtype=np.float32),
                  np.ones((1, HC), dtype=np.float32))      # [E, E*HC]

    in_maps = []
    for c in range(NCORES):
        sl = slice(c * OC, (c + 1) * OC)
        # w1[i_lo, k, h, e, hc] = dense_w[e, c*OC + h*64 + hc, k*128 + i_lo]
        w1 = np.ascontiguousarray(
            (dense_w[:, sl, :] * WSCALE)        # [E, OC, H], fp8-ranged
            .reshape(E, 2, HC, KT, P)           # [e, h, hc, k, i_lo]
            .transpose(4, 3, 1, 0, 2)           # [i_lo, k, h, e, hc]
        ).astype(ml_dtypes.float8_e3m4)

        # db row: dense_b[e, c*OC + h*64 + hc] laid as (h, e, hc); row 0
        # only.  Carries WSCALE so it accumulates into the scaled psum.
        dbrow = np.zeros((E, 2 * E * HC), dtype=np.float32)
        dbrow[0] = (dense_b[:, sl].reshape(E, 2, HC)
                    .transpose(1, 0, 2).reshape(-1)) * WSCALE

        # ow2[e, (h, l, hc)] = out_w[e, l, c*OC + h*64 + hc]
        ow2 = (out_w[:, :, sl].reshape(E, L, 2, HC)
               .transpose(0, 2, 1, 3).reshape(E, L * OC))
        ob = out_b if c == 0 else np.zeros_like(out_b)
        cp = np.ascontiguousarray(
            np.concatenate([gt, ow2, ob, gtz, gtdup, ebc, dbrow], axis=1)
        ).astype(BF)
        in_maps.append({
            "xt": xt,
            "w1": w1,
            "cp": cp,
        })
    return in_maps


def _run(in_maps, trace=False, tmpdir=None):
    global _cached
    from concourse.bass_utils import run_bass_kernel_spmd

    if _cached is None:
        _cached = _build()
    res = run_bass_kernel_spmd(
        _cached, in_maps, list(range(NCORES)), trace=trace, tmpdir=tmpdir,
    )
    return res


def kernel(X, gates, dense_w, dense_b, out_w, out_b):
    in_maps = _prep_inputs(X, gates, dense_w, dense_b, out_w, out_b)
    res = _run(in_maps)
    acc = np.zeros((B, L), dtype=np.float64)
    for c in range(NCORES):
        part = res.results[c]["out"].astype(np.float64)   # [L, 128]
        acc += part.T.reshape(2, B, L).sum(axis=0)
    return acc.astype(np.float32)
